# revision 17
# baseline (speedup 1.0000x reference)
"""Haar wavelet frequency extractor — Trainium2 Bass kernel (bf16 I/O).

Math: for each 2x2 block [[a,b],[c,d]] of x the reference computes the
orthonormal Haar decomposition, then reconstructs a low-pass image (LL
only) and a high-pass image (LH+HL+HH).  The four filters are an
orthonormal basis of R^4, so x_low + x_high == x exactly and

    x_low[2i+p, 2j+q] = 0.25 * (a + b + c + d)   (block mean, broadcast 2x2)
    x_high = x - x_low

Pure memory-bound.  fp32 I/O needs 96 MiB of HBM traffic per core and
measured 235 us — ~98% of the ~435 GB/s 16-DMA-engine per-core ceiling.
So all device I/O is bf16 (quantization adds ~3e-3 relative l2 error,
well inside the 2e-2 gate), halving traffic to 48 MiB per core.

Layout: DVE perf modes require dense step-1 access — strided (par, c)
slicing runs at 1x, which made a naive bf16 kernel vector-bound.  The
host therefore de-interleaves the 2x2 block structure when casting to
bf16 (pure relayout, no arithmetic): per chunk of CI images each SBUF
partition holds the four block planes contiguously, free index =
((par*2 + c)*CI + img)*512 + r*256 + w2 for image row 4p + 2r + par,
column 2*w2 + c.  Every engine op is then a fully contiguous slab.

x_low is never materialized in SBUF: all four planes of a chunk's x_low
equal the block-mean tile mt, so the low-store is four DMAs that each
read mt and write one DRAM plane slab (dense 4 KiB descriptors).

Engine split (per chunk):
  DVE : vs = planes[par0] + planes[par1]; sv = vs[c0] + vs[c1];
        mt = 0.25*sv (bf16);  hig plane(par,c) = x plane - mt  (4 subs)
  SP  : input DMAs + lagged 4-plane low-store DMAs from mt
  ACT : high-store DMAs

TRN2 hazard note: DMA issues execute on an engine's *sequencer* while
compute runs in the *engine* pipe with late writeback — a DMA reading an
engine's output must be gated on that output's then_inc semaphore, not
just program order.

Raw Bass (not Tile): DMAs are gated by standalone wait_ge instructions.
The four concurrent low-stores of a chunk share one slot semaphore
(+16 each, +64 per chunk), which stays unambiguous because reuse waits
only on full-chunk multiples.
"""

from contextlib import ExitStack

import ml_dtypes
import numpy as np

import concourse.bass as bass
import concourse.mybir as mybir
from concourse.bass_utils import run_bass_kernel_spmd

BF16 = mybir.dt.bfloat16
NP_BF16 = ml_dtypes.bfloat16
N_CORES = 8
B, C, H, W = 4, 64, 512, 512
N_IMG = (B * C) // N_CORES  # 32 images per core
P = 128                     # SBUF partitions
FREE = (H // P) * W         # 2048 elems per partition per image

CI = 4                      # images per chunk
NCH = N_IMG // CI           # chunks per core
CF = CI * FREE              # free elems per partition per chunk (8192)
PL = CF // 4                # plane size: (par, c) plane of a chunk (2048)
S = 4                       # pipeline slots
L = 2                       # low-store lag (chunks) on the SP ring

_NC = None


def _build(nch: int = NCH, detect_races: bool = False):
    nc = bass.Bass(detect_race_conditions=detect_races)
    x = nc.dram_tensor("x", [nch, P, CF], BF16, kind="ExternalInput")
    xl = nc.dram_tensor("x_low", [nch, P, CF], BF16, kind="ExternalOutput")
    xh = nc.dram_tensor("x_high", [nch, P, CF], BF16, kind="ExternalOutput")

    with ExitStack() as st:
        xin = [st.enter_context(nc.sbuf_tensor(f"xin{s}", [P, CF], BF16))
               for s in range(S)]
        hig = [st.enter_context(nc.sbuf_tensor(f"hig{s}", [P, CF], BF16))
               for s in range(S)]
        mt = [st.enter_context(nc.sbuf_tensor(f"mt{s}", [P, PL], BF16))
              for s in range(S)]
        # DVE-private intermediates: single buffers, in-order engine
        vsm = st.enter_context(nc.sbuf_tensor("vsm", [P, CF // 2], BF16))
        svm = st.enter_context(nc.sbuf_tensor("svm", [P, PL], BF16))
        ld = [st.enter_context(nc.semaphore(f"ld{s}")) for s in range(S)]
        stl = [st.enter_context(nc.semaphore(f"stl{s}")) for s in range(S)]
        sth = [st.enter_context(nc.semaphore(f"sth{s}")) for s in range(S)]
        dve_sv = st.enter_context(nc.semaphore("dve_sv"))    # means ready
        dve_sub = st.enter_context(nc.semaphore("dve_sub"))  # highs ready

        # allocating a semaphore does NOT clear it; values persist across
        # NEFF executions of a loaded model — clear ours before any use.
        allsems = [*ld, *stl, *sth, dve_sv, dve_sub]
        nums = sorted(h.num for h in allsems)
        assert nums == list(range(nums[0], nums[-1] + 1))
        nc.gpsimd.sem_clear(range(nums[0], nums[-1] + 1))
        nc.all_engine_barrier()

        blk = st.enter_context(nc.Block())

        # SP ring: loads + lagged low-plane stores straight from mt
        @blk.sync
        def _(sync):
            def store_low(j):
                sj = j % S
                sync.wait_ge(dve_sv, j + 1)
                for pl in range(4):
                    sync.dma_start(out=xl[j][:, pl * PL:(pl + 1) * PL],
                                   in_=mt[sj][:, :]).then_inc(stl[sj], 16)

            for k in range(nch):
                s = k % S
                if k >= S:
                    # xin slot free once the DVE subs of chunk k-S ran
                    sync.wait_ge(dve_sub, k - S + 1)
                sync.dma_start(out=xin[s][:, :], in_=x[k]
                               ).then_inc(ld[s], 16)
                if k >= L:
                    store_low(k - L)
            for j in range(nch - L, nch):
                store_low(j)

        # DVE: block sums, means, and the four high planes — all dense
        @blk.vector
        def _(vector):
            for i in range(nch):
                s = i % S
                vector.wait_ge(ld[s], 16 * (i // S + 1))
                xi = xin[s]
                vector.tensor_add(vsm[:, :], xi[:, 0:CF // 2],
                                  xi[:, CF // 2:CF])
                vector.tensor_add(svm[:, :], vsm[:, 0:PL], vsm[:, PL:2 * PL])
                if i >= S:
                    # mt slot free once all 4 low-stores of chunk i-S ran
                    vector.wait_ge(stl[s], 64 * (i // S))
                vector.tensor_scalar_mul(mt[s][:, :], svm[:, :], 0.25
                                         ).then_inc(dve_sv, 1)
                if i >= S:
                    vector.wait_ge(sth[s], 16 * (i // S))
                for pl in range(4):
                    ins = vector.tensor_sub(
                        hig[s][:, pl * PL:(pl + 1) * PL],
                        xi[:, pl * PL:(pl + 1) * PL], mt[s][:, :])
                    if pl == 3:
                        ins.then_inc(dve_sub, 1)

        # ACT: high stores only
        @blk.scalar
        def _(scalar):
            for j in range(nch):
                sj = j % S
                scalar.wait_ge(dve_sub, j + 1)
                scalar.dma_start(out=xh[j], in_=hig[sj][:, :]
                                 ).then_inc(sth[sj], 16)

    return nc


def _get_nc():
    global _NC
    if _NC is None:
        _NC = _build()
    return _NC


# host <-> device layout: [core, chunk, p, par, c, img, r, w2] on device
def _shard(x):
    xv = x.reshape(N_CORES, NCH, CI, P, 2, 2, 256, 2)
    #              core  chunk img  p   r  par w2  c
    return (xv.transpose(0, 1, 3, 5, 7, 2, 4, 6)
            .astype(NP_BF16)
            .reshape(N_CORES, NCH, P, CF))


def _unshard(y):
    yv = y.reshape(N_CORES, NCH, P, 2, 2, CI, 2, 256)
    #              core  chunk p  par c  img  r  w2
    return (yv.transpose(0, 1, 5, 2, 6, 3, 7, 4)
            .astype(np.float32)
            .reshape(B, C, H, W))


def kernel(x: np.ndarray):
    x = np.asarray(x)
    assert x.shape == (B, C, H, W)
    xb = _shard(x)
    in_maps = [{"x": xb[c]} for c in range(N_CORES)]
    res = run_bass_kernel_spmd(_get_nc(), in_maps,
                               core_ids=list(range(N_CORES)))
    low = np.stack([res.results[c]["x_low"] for c in range(N_CORES)])
    high = np.stack([res.results[c]["x_high"] for c in range(N_CORES)])
    return _unshard(low), _unshard(high)


# revision 18
# speedup vs baseline: 1.0313x; 1.0313x over previous
"""Haar wavelet frequency extractor — Trainium2 Bass kernel (bf16 I/O).

Math: for each 2x2 block [[a,b],[c,d]] of x the reference computes the
orthonormal Haar decomposition, then reconstructs a low-pass image (LL
only) and a high-pass image (LH+HL+HH).  The four filters are an
orthonormal basis of R^4, so x_low + x_high == x exactly and

    x_low[2i+p, 2j+q] = 0.25 * (a + b + c + d)   (block mean, broadcast 2x2)
    x_high = x - x_low

Pure memory-bound.  fp32 I/O needs 96 MiB of HBM traffic per core and
measured 235 us — ~98% of the ~435 GB/s 16-DMA-engine per-core ceiling.
So all device I/O is bf16 (quantization adds ~3e-3 relative l2 error,
well inside the 2e-2 gate), halving traffic to 48 MiB per core.

Layout: DVE perf modes require dense step-1 access — strided (par, c)
slicing runs at 1x, which made a naive bf16 kernel vector-bound.  The
host therefore de-interleaves the 2x2 block structure when casting to
bf16 (pure relayout, no arithmetic): per chunk of CI images each SBUF
partition holds the four block planes contiguously, free index =
((par*2 + c)*CI + img)*512 + r*256 + w2 for image row 4p + 2r + par,
column 2*w2 + c.  Every engine op is then a fully contiguous slab.

x_low is never materialized in SBUF: all four planes of a chunk's x_low
equal the block-mean tile mt, so the low-store is four DMAs that each
read mt and write one DRAM plane slab (dense 4 KiB descriptors).

Engine split (per chunk):
  DVE : vs = planes[par0] + planes[par1]; sv = vs[c0] + vs[c1];
        mt = 0.25*sv (bf16);  hig plane(par,c) = x plane - mt  (4 subs)
  SP  : input DMAs + lagged 4-plane low-store DMAs from mt
  ACT : high-store DMAs

TRN2 hazard note: DMA issues execute on an engine's *sequencer* while
compute runs in the *engine* pipe with late writeback — a DMA reading an
engine's output must be gated on that output's then_inc semaphore, not
just program order.

Raw Bass (not Tile): DMAs are gated by standalone wait_ge instructions.
The four concurrent low-stores of a chunk share one slot semaphore
(+16 each, +64 per chunk), which stays unambiguous because reuse waits
only on full-chunk multiples.
"""

from contextlib import ExitStack

import ml_dtypes
import numpy as np

import concourse.bass as bass
import concourse.mybir as mybir
from concourse.bass_utils import run_bass_kernel_spmd

BF16 = mybir.dt.bfloat16
NP_BF16 = ml_dtypes.bfloat16
N_CORES = 8
B, C, H, W = 4, 64, 512, 512
N_IMG = (B * C) // N_CORES  # 32 images per core
P = 128                     # SBUF partitions
FREE = (H // P) * W         # 2048 elems per partition per image

CI = 4                      # images per chunk
NCH = N_IMG // CI           # chunks per core
CF = CI * FREE              # free elems per partition per chunk (8192)
PL = CF // 4                # plane size: (par, c) plane of a chunk (2048)
S = 4                       # pipeline slots
L = 2                       # low-store lag (chunks) on the SP ring

_NC = None


def _build(nch: int = NCH, detect_races: bool = False):
    nc = bass.Bass(detect_race_conditions=detect_races)
    x = nc.dram_tensor("x", [nch, P, CF], BF16, kind="ExternalInput")
    xl = nc.dram_tensor("x_low", [nch, P, CF], BF16, kind="ExternalOutput")
    xh = nc.dram_tensor("x_high", [nch, P, CF], BF16, kind="ExternalOutput")

    with ExitStack() as st:
        xin = [st.enter_context(nc.sbuf_tensor(f"xin{s}", [P, CF], BF16))
               for s in range(S)]
        hig = [st.enter_context(nc.sbuf_tensor(f"hig{s}", [P, CF], BF16))
               for s in range(S)]
        mt = [st.enter_context(nc.sbuf_tensor(f"mt{s}", [P, PL], BF16))
              for s in range(S)]
        # DVE-private intermediates: single buffers, in-order engine
        vsm = st.enter_context(nc.sbuf_tensor("vsm", [P, CF // 2], BF16))
        svm = st.enter_context(nc.sbuf_tensor("svm", [P, PL], BF16))
        ld = [st.enter_context(nc.semaphore(f"ld{s}")) for s in range(S)]
        stl = [st.enter_context(nc.semaphore(f"stl{s}")) for s in range(S)]
        sth = [st.enter_context(nc.semaphore(f"sth{s}")) for s in range(S)]
        dve_sv = st.enter_context(nc.semaphore("dve_sv"))    # means ready
        dve_sub = st.enter_context(nc.semaphore("dve_sub"))  # highs ready

        # allocating a semaphore does NOT clear it; values persist across
        # NEFF executions of a loaded model — clear ours before any use.
        allsems = [*ld, *stl, *sth, dve_sv, dve_sub]
        nums = sorted(h.num for h in allsems)
        assert nums == list(range(nums[0], nums[-1] + 1))
        nc.gpsimd.sem_clear(range(nums[0], nums[-1] + 1))
        nc.all_engine_barrier()

        blk = st.enter_context(nc.Block())

        # SP ring: loads only — never stalls behind store gating
        @blk.sync
        def _(sync):
            for k in range(nch):
                s = k % S
                if k >= S:
                    # xin slot free once the DVE subs of chunk k-S ran
                    sync.wait_ge(dve_sub, 4 * (k - S + 1))
                sync.dma_start(out=xin[s][:, :], in_=x[k]
                               ).then_inc(ld[s], 16)

        # DVE: block sums, means, and the four high planes — all dense
        @blk.vector
        def _(vector):
            for i in range(nch):
                s = i % S
                vector.wait_ge(ld[s], 16 * (i // S + 1))
                xi = xin[s]
                vector.tensor_add(vsm[:, :], xi[:, 0:CF // 2],
                                  xi[:, CF // 2:CF])
                vector.tensor_add(svm[:, :], vsm[:, 0:PL], vsm[:, PL:2 * PL])
                if i >= S:
                    # mt slot free once all 4 low-stores of chunk i-S ran
                    vector.wait_ge(stl[s], 64 * (i // S))
                vector.tensor_scalar_mul(mt[s][:, :], svm[:, :], 0.25
                                         ).then_inc(dve_sv, 1)
                if i >= S:
                    # hig slot free once all 4 high-stores of chunk i-S ran
                    vector.wait_ge(sth[s], 64 * (i // S))
                for pl in range(4):
                    vector.tensor_sub(
                        hig[s][:, pl * PL:(pl + 1) * PL],
                        xi[:, pl * PL:(pl + 1) * PL], mt[s][:, :]
                    ).then_inc(dve_sub, 1)

        # ACT: all stores.  Low planes of chunk j as soon as its means are
        # ready, then the high planes of chunk j-1 one plane at a time so
        # each 0.5 MiB store issues right after its sub retires.
        @blk.scalar
        def _(scalar):
            def store_high(j):
                sj = j % S
                for pl in range(4):
                    scalar.wait_ge(dve_sub, 4 * j + pl + 1)
                    scalar.dma_start(
                        out=xh[j][:, pl * PL:(pl + 1) * PL],
                        in_=hig[sj][:, pl * PL:(pl + 1) * PL]
                    ).then_inc(sth[sj], 16)

            for j in range(nch):
                sj = j % S
                scalar.wait_ge(dve_sv, j + 1)
                for pl in range(4):
                    scalar.dma_start(out=xl[j][:, pl * PL:(pl + 1) * PL],
                                     in_=mt[sj][:, :]).then_inc(stl[sj], 16)
                if j >= 1:
                    store_high(j - 1)
            store_high(nch - 1)

    return nc


def _get_nc():
    global _NC
    if _NC is None:
        _NC = _build()
    return _NC


# host <-> device layout: [core, chunk, p, par, c, img, r, w2] on device
def _shard(x):
    xv = x.reshape(N_CORES, NCH, CI, P, 2, 2, 256, 2)
    #              core  chunk img  p   r  par w2  c
    return (xv.transpose(0, 1, 3, 5, 7, 2, 4, 6)
            .astype(NP_BF16)
            .reshape(N_CORES, NCH, P, CF))


def _unshard(y):
    yv = y.reshape(N_CORES, NCH, P, 2, 2, CI, 2, 256)
    #              core  chunk p  par c  img  r  w2
    return (yv.transpose(0, 1, 5, 2, 6, 3, 7, 4)
            .astype(np.float32)
            .reshape(B, C, H, W))


def kernel(x: np.ndarray):
    x = np.asarray(x)
    assert x.shape == (B, C, H, W)
    xb = _shard(x)
    in_maps = [{"x": xb[c]} for c in range(N_CORES)]
    res = run_bass_kernel_spmd(_get_nc(), in_maps,
                               core_ids=list(range(N_CORES)))
    low = np.stack([res.results[c]["x_low"] for c in range(N_CORES)])
    high = np.stack([res.results[c]["x_high"] for c in range(N_CORES)])
    return _unshard(low), _unshard(high)
